# revision 13
# baseline (speedup 1.0000x reference)
"""Multi-head causal attention on 8 Trainium2 NeuronCores.

Problem: x[4,2048,1024] @ {W_q,W_k,W_v}, 16 heads x d_k=64, causal softmax,
context @ W_o. Sharding: 8 cores = 4 batches x 2 head-groups (tensor
parallel over heads, data parallel over batch). Each core computes, for its
batch b and its 8 heads: projections, causal attention, and a partial
output  context_g @ W_o[g-rows]  [2048,1024]. Host sums the two partials
per batch (the W_o row-split reduction) and stacks batches.

v2 (bf16): all matmul operands in bf16 (inputs host-cast). Wins vs the
f32r baseline: FWL fast weight loads (LDWEIGHTS was 223us), exact causal
trims (no 256-column floor), 1-cycle/row transposes, halved DMA + SBUF so
the out-projection lhsT (ctx_l) is resident from the start (no DRAM
scratch roundtrip). Score matmuls are packed two-heads-at-a-time: the
even head lives on PE row-groups 0-1 (partitions 0-63) and the odd head
on row-groups 2-3 (partitions 64-127); issued adjacently they execute
concurrently, halving the score phase. Ctx keeps the M=65 ones-row trick
(l rides along free) - col-packing would lose the denominator.

Layout (contraction-major; single x transpose):
  xT[D,S]     PE-transpose of x (bf16, 8 chunks per sb-block in one bank)
  QT/KT[dd,S] = W.T x.T    V[S,dd] + ones column per head
  ST[k,q]     pair PSUM [128k, 2head, 512q]; E = exp(ST/8) one ACT per
              k-block covering both heads; diag mask = fixed 128-wide
              triangle (base=0, j>=k) on gpsimd
  ctxT[65,q]  accumulated over k-blocks per head; staged to SBUF bf16
              immediately (frees PSUM), then scaled by 1/l (partition-
              spread DVE reciprocal + gpsimd broadcast) into resident
              ctx_l
  out[q,1024] = sum_c ctx_l chunks @ W_o, streamed per 128-row group

Schedule: ACT (exp) throughput paces attention, so fill work is woven
between attention groups: projections for quarter qt+1 and the output
projection for quarter qt-1 both interleave into attention quarter qt.
"""
import numpy as np

import concourse.bacc as bacc
import concourse.mybir as mybir
import concourse.tile as tile
from concourse.bass_utils import run_bass_kernel_spmd
from concourse.masks import make_identity

P = 128
S = 2048
D = 1024
GW = 512          # per-core head-group width (8 heads x 64)
DK = 64
HG = 8
NHP = HG // 2     # head pairs (even head rows 0-63, odd head rows 64-127)
NDC = D // P
NQT = S // 512
NSB = S // P
NCH = GW // P

F32 = mybir.dt.float32
BF = mybir.dt.bfloat16
SCALE = 0.125
N_CORES = 8


def vstart(kb, qt):
    # first causally-valid q in the 512-wide query tile for k-block kb
    return min(max(0, P * (kb - 4 * qt)), 384)


def build():
    nc = bacc.Bacc("TRN2", target_bir_lowering=False, debug=False)
    xb = nc.dram_tensor("xb", [S, D], BF, kind="ExternalInput")
    wq = nc.dram_tensor("wq", [D, GW], BF, kind="ExternalInput")
    wk = nc.dram_tensor("wk", [D, GW], BF, kind="ExternalInput")
    wv = nc.dram_tensor("wv", [D, GW], BF, kind="ExternalInput")
    wo = nc.dram_tensor("wo", [GW, D], BF, kind="ExternalInput")
    outp = nc.dram_tensor("outp", [S, D], BF, kind="ExternalOutput")

    with tile.TileContext(nc) as tc, \
         tc.tile_pool(name="const", bufs=1) as cpool, \
         tc.tile_pool(name="stores", bufs=1) as stores, \
         tc.tile_pool(name="wqkv", bufs=1) as wpool, \
         tc.tile_pool(name="xin", bufs=2) as xin, \
         tc.tile_pool(name="xt", bufs=2) as xtp, \
         tc.tile_pool(name="qtp", bufs=2) as qtp, \
         tc.tile_pool(name="e", bufs=24) as epool, \
         tc.tile_pool(name="lwork", bufs=1) as lwork, \
         tc.tile_pool(name="cstage", bufs=3) as cstage, \
         tc.tile_pool(name="ostage", bufs=3) as ostage, \
         tc.tile_pool(name="ps_sc", bufs=2, space="PSUM") as ps_sc, \
         tc.tile_pool(name="ps_cx", bufs=2, space="PSUM") as ps_cx, \
         tc.tile_pool(name="ps_pj", bufs=2, space="PSUM") as ps_pj:

        ident = cpool.tile([P, P], BF, tag="ident")
        make_identity(nc, ident[:])

        kT = stores.tile([P, NCH, S], BF, tag="kT")
        v_aug = stores.tile([P, NSB, HG, DK + 1], BF, tag="v")
        nc.vector.tensor_copy(
            v_aug[:, :, :, DK:DK + 1],
            nc.const_aps.tensor(1.0, (P, NSB, HG, 1), F32))
        ctx_l = stores.tile([P, NCH, S], BF, tag="ctxl")
        wo_t = stores.tile([P, NCH, D], BF, tag="wo")
        qT_tiles = {}
        xt_cur = {}

        # ---- projection emission units for one sequence-quarter ----------
        pstate = {}

        def load_wqkv():
            # on the ACT hwdge queue: parallel with x loads on sync
            wq_t = wpool.tile([P, NDC, GW], BF, tag="wq")
            wk_t = wpool.tile([P, NDC, GW], BF, tag="wk")
            wv_t = wpool.tile([P, NDC, GW], BF, tag="wv")
            for j in range(NCH):
                nc.scalar.dma_start(
                    wk_t[:, :, j * P:(j + 1) * P],
                    wk[:, j * P:(j + 1) * P].rearrange("(dc p) n -> p dc n", p=P))
            for j in range(NCH):
                nc.scalar.dma_start(
                    wq_t[:, :, j * P:(j + 1) * P],
                    wq[:, j * P:(j + 1) * P].rearrange("(dc p) n -> p dc n", p=P))
            nc.scalar.dma_start(wv_t[:], wv.rearrange("(dc p) n -> p dc n", p=P))
            pstate["w"] = (wq_t, wk_t, wv_t)

        def load_wo():
            # deferred: not needed until the first out-projection (qt=1),
            # keeps the startup off the HBM critical path
            nc.scalar.dma_start(wo_t[:], wo.rearrange("(c p) n -> p c n", p=P))

        def p_start(q4):
            xt_cur[q4] = xtp.tile([P, NDC, 512], BF, tag="xt", name=f"xt{q4}")
            qT_tiles[q4] = qtp.tile([P, NCH, 512], BF, tag="qT", name=f"qT{q4}")

        def transpose_block(q4, sbl):
            xt_q = xt_cur[q4]
            sb = q4 * 4 + sbl
            x_blk = xin.tile([P, D], BF, tag="xin")
            nc.sync.dma_start(x_blk[:], xb[sb * P:(sb + 1) * P, :])
            tp_ps = ps_pj.tile([P, NDC, P], BF, tag="pj")
            for dc in range(NDC):
                nc.tensor.transpose(
                    tp_ps[:, dc, :], x_blk[:, dc * P:(dc + 1) * P], ident[:])
            nc.vector.tensor_copy(
                xt_q[:, :, sbl * P:(sbl + 1) * P], tp_ps[:])

        def qk_proj(q4, w_i, j, xt_q):
            w_t = pstate["w"][w_i]
            dst = qT_tiles[q4] if w_i == 0 else kT
            pj = ps_pj.tile([P, 512], F32, tag="pj")
            for dc in range(NDC):
                nc.tensor.matmul(pj[:], w_t[:, dc, j * P:(j + 1) * P],
                                 xt_q[:, dc, :],
                                 start=(dc == 0), stop=(dc == NDC - 1))
            if w_i == 0:
                nc.vector.tensor_copy(dst[:, j, :], pj[:])
            else:
                nc.vector.tensor_copy(
                    dst[:, j, q4 * 512:(q4 + 1) * 512], pj[:])

        def v_proj(q4, sbl, xt_q):
            sb = q4 * 4 + sbl
            pj = ps_pj.tile([P, 512], F32, tag="pj")
            for dc in range(NDC):
                nc.tensor.matmul(pj[:], xt_q[:, dc, sbl * P:(sbl + 1) * P],
                                 pstate["w"][2][:, dc, :],
                                 start=(dc == 0), stop=(dc == NDC - 1))
            nc.vector.tensor_copy(v_aug[:, sb, :, :DK], pj[:])

        def proj_units(q4):
            # interleaved K/Q/V per output chunk so woven consumers (scores
            # of head-pair j, ctx needing V) unblock as early as possible
            units = [lambda: p_start(q4)]
            units += [lambda sbl=sbl: transpose_block(q4, sbl) for sbl in range(4)]

            def mk(w_i, j):
                return lambda: qk_proj(q4, w_i, j, xt_cur[q4])

            def mkv(sbl):
                return lambda: v_proj(q4, sbl, xt_cur[q4])
            units += [mk(1, 0), mk(0, 0), mk(1, 1), mk(0, 1), mkv(0), mkv(1),
                      mk(1, 2), mk(0, 2), mk(1, 3), mk(0, 3), mkv(2), mkv(3)]
            return units

        # ---- attention group emitters (two heads 2hp, 2hp+1 at once) -----
        def emit_scores(hp, qt):
            e_blocks = []
            for kb in range(4 * (qt + 1)):
                vs = vstart(kb, qt)
                s_ps = ps_sc.tile([P, 2, 512], F32, tag="sc")
                for slot in range(2):
                    po = 64 * slot
                    nc.tensor.matmul(s_ps[:, slot, vs:],
                                     kT[po:po + 64, hp, kb * P:(kb + 1) * P],
                                     qT_tiles[qt][po:po + 64, hp, vs:],
                                     start=True, stop=True)
                e_sb = epool.tile([P, 2, 512], BF, tag="e")
                nc.scalar.activation(e_sb[:, :, vs:], s_ps[:, :, vs:],
                                     mybir.ActivationFunctionType.Exp,
                                     scale=SCALE)
                if kb >= 4 * qt:
                    # diagonal block: zero below-diagonal; in the 128-wide
                    # window starting at vs the mask is always keep j >= k
                    nc.gpsimd.affine_select(
                        out=e_sb[:, :, vs:vs + P], in_=e_sb[:, :, vs:vs + P],
                        compare_op=mybir.AluOpType.is_ge,
                        fill=0.0, base=0,
                        pattern=[[0, 2], [1, P]], channel_multiplier=-1)
                e_blocks.append((e_sb, vs))
            return e_blocks

        def emit_ctx(hp, qt, e_blocks):
            nk = 4 * (qt + 1)
            ctx_ps = [ps_cx.tile([P, 512], F32, tag="cx", name=f"cx{slot}")
                      for slot in range(2)]
            for kb in range(nk):
                e_sb, vs = e_blocks[kb]
                for slot in range(2):
                    nc.tensor.matmul(ctx_ps[slot][0:DK + 1, vs:],
                                     v_aug[:, kb, 2 * hp + slot, :],
                                     e_sb[:, slot, vs:],
                                     start=(kb == 0), stop=(kb == nk - 1),
                                     skip_group_check=True)
            for slot in range(2):
                po = 64 * slot
                # stage ctx+l out of PSUM immediately to free the bank
                stg = cstage.tile([P, 512], BF, tag="stg")
                nc.vector.tensor_copy(stg[0:DK + 1, :], ctx_ps[slot][0:DK + 1, :])
                # 1/l: spread the 512 l values over 8 partitions so the DVE
                # reciprocal (serial within a partition) is cheap, gather
                # back, broadcast over the DK ctx partitions.
                lsp = lwork.tile([P, 512], BF, tag="lsp")
                nc.sync.dma_start(lsp[0:8, 0:64], stg[DK:DK + 1, :])
                lre = lwork.tile([P, 512], F32, tag="lre")
                nc.vector.reciprocal(lre[0:8, 0:64], lsp[0:8, 0:64])
                linv = lwork.tile([P, 512], F32, tag="linv")
                nc.sync.dma_start(linv[0:1, :], lre[0:8, 0:64])
                lrep = lwork.tile([P, 512], F32, tag="lrep")
                nc.gpsimd.partition_broadcast(lrep[0:DK, :], linv[0:1, :],
                                              channels=DK)
                nc.vector.tensor_mul(
                    out=ctx_l[po:po + DK, hp, qt * 512:(qt + 1) * 512],
                    in0=stg[0:DK, :], in1=lrep[0:DK, :])

        def out_group(qb, nh):
            po_ps = ps_pj.tile([P, 512], F32, tag="pj")
            for c in range(NCH):
                nc.tensor.matmul(
                    po_ps[:], ctx_l[:, c, qb * P:(qb + 1) * P],
                    wo_t[:, c, nh * 512:(nh + 1) * 512],
                    start=(c == 0), stop=(c == NCH - 1))
            ost = ostage.tile([P, 512], BF, tag="ost")
            nc.vector.tensor_copy(ost[:], po_ps[:])
            nc.sync.dma_start(
                outp[qb * P:(qb + 1) * P, nh * 512:(nh + 1) * 512],
                ost[:])

        # ---- interleaved emission ----------------------------------------
        # Startup prefix: only what the first attention group needs — x
        # quarter 0 transposed, W_k/W_q/W_v in flight, K/Q chunk 0
        # projected. The rest of quarter 0's projections weave into
        # attention quarter 0 alongside quarter 1's.
        p0 = proj_units(0)
        for u in p0[:5]:
            u()                      # p_start + 4 transpose blocks
        load_wqkv()
        p0[5]()                      # K proj chunk 0
        p0[6]()                      # Q proj chunk 0
        # emission order is dependency-binding order: all V units must be
        # emitted before the first emit_ctx (slot hp1), K/Q chunk j before
        # scores of head-pair j
        p0_rest = [p0[9], p0[10], p0[15], p0[16],          # V0..V3
                   p0[7], p0[8], p0[11], p0[12], p0[13], p0[14]]  # K/Q 1..3

        prev = None
        for qt in range(NQT):
            pu = list(p0_rest) if qt == 0 else []
            p0_rest = []
            if qt < NQT - 1:
                pu += proj_units(qt + 1)
            if qt == 0:
                pu.append(load_wo)
            held = []
            if qt >= 1:
                # output projection for quarter qt-1 (ctx complete once the
                # last head-pair of qt-1 is emitted, at hp=0 below); at
                # qt=3 hold half back as PE filler for the final 1/l chain
                oq = qt - 1
                og = [(lambda qb=qb, nh=nh: out_group(qb, nh))
                      for qb in range(4 * oq, 4 * oq + 4) for nh in range(2)]
                if qt == NQT - 1:
                    pu += og[:4]
                    held = og[4:]
                else:
                    pu += og
            pi = 0
            for hp in range(NHP):
                e_blocks = emit_scores(hp, qt)
                if prev is not None:
                    emit_ctx(*prev)
                prev = (hp, qt, e_blocks)
                take = ((hp + 1) * len(pu)) // NHP - (hp * len(pu)) // NHP
                for _ in range(take):
                    pu[pi]()
                    pi += 1
        emit_ctx(*prev)
        for u in held:               # out-proj qt=2 leftovers hide the chain
            u()
        for qb in range(S // P - 4, S // P):   # out-proj for the last quarter
            for nh in range(2):
                out_group(qb, nh)
    nc.compile()
    return nc


_NC_CACHE = None


def _get_nc():
    global _NC_CACHE
    if _NC_CACHE is None:
        _NC_CACHE = build()
    return _NC_CACHE


def _run(x, W_q, W_k, W_v, W_o, trace=False, tmpdir=None):
    import ml_dtypes
    bf16 = ml_dtypes.bfloat16
    x = np.asarray(x).astype(bf16)
    W_q = np.asarray(W_q).astype(bf16)
    W_k = np.asarray(W_k).astype(bf16)
    W_v = np.asarray(W_v).astype(bf16)
    W_o = np.asarray(W_o).astype(bf16)
    B = x.shape[0]
    in_maps = []
    for c in range(N_CORES):
        b, g = c // 2, c % 2
        in_maps.append({
            "xb": np.ascontiguousarray(x[b]),
            "wq": np.ascontiguousarray(W_q[:, g * GW:(g + 1) * GW]),
            "wk": np.ascontiguousarray(W_k[:, g * GW:(g + 1) * GW]),
            "wv": np.ascontiguousarray(W_v[:, g * GW:(g + 1) * GW]),
            "wo": np.ascontiguousarray(W_o[g * GW:(g + 1) * GW, :]),
        })
    nc = _get_nc()
    res = run_bass_kernel_spmd(nc, in_maps, core_ids=list(range(N_CORES)),
                               trace=trace, tmpdir=tmpdir)
    out = np.empty((B, S, D), np.float32)
    for b in range(B):
        out[b] = (res.results[2 * b]["outp"].astype(np.float32)
                  + res.results[2 * b + 1]["outp"].astype(np.float32))
    return out, res


def kernel(x, W_q, W_k, W_v, W_o):
    out, _ = _run(x, W_q, W_k, W_v, W_o)
    return out


# revision 21
# speedup vs baseline: 1.0091x; 1.0091x over previous
"""Multi-head causal attention on 8 Trainium2 NeuronCores.

Problem: x[4,2048,1024] @ {W_q,W_k,W_v}, 16 heads x d_k=64, causal softmax,
context @ W_o. Sharding: 8 cores = 4 batches x 2 head-groups (tensor
parallel over heads, data parallel over batch). Each core computes, for its
batch b and its 8 heads: projections, causal attention, and a partial
output  context_g @ W_o[g-rows]  [2048,1024]. Host sums the two partials
per batch (the W_o row-split reduction) and stacks batches.

v2 (bf16): all matmul operands in bf16 (inputs host-cast). Wins vs the
f32r baseline: FWL fast weight loads (LDWEIGHTS was 223us), exact causal
trims (no 256-column floor), 1-cycle/row transposes, halved DMA + SBUF so
the out-projection lhsT (ctx_l) is resident from the start (no DRAM
scratch roundtrip). Score matmuls are packed two-heads-at-a-time: the
even head lives on PE row-groups 0-1 (partitions 0-63) and the odd head
on row-groups 2-3 (partitions 64-127); issued adjacently they execute
concurrently, halving the score phase. Ctx keeps the M=65 ones-row trick
(l rides along free) - col-packing would lose the denominator.

Layout (contraction-major; single x transpose):
  xT[D,S]     PE-transpose of x (bf16, 8 chunks per sb-block in one bank)
  QT/KT[dd,S] = W.T x.T    V[S,dd] + ones column per head
  ST[k,q]     pair PSUM [128k, 2head, 512q]; E = exp(ST/8) one ACT per
              k-block covering both heads; diag mask = fixed 128-wide
              triangle (base=0, j>=k) on gpsimd
  ctxT[65,q]  accumulated over k-blocks per head; staged to SBUF bf16
              immediately (frees PSUM), then scaled by 1/l (partition-
              spread DVE reciprocal + gpsimd broadcast) into resident
              ctx_l
  out[q,1024] = sum_c ctx_l chunks @ W_o, streamed per 128-row group

Schedule: ACT (exp) throughput paces attention, so fill work is woven
between attention groups: projections for quarter qt+1 and the output
projection for quarter qt-1 both interleave into attention quarter qt.
"""
import numpy as np

import concourse.bacc as bacc
import concourse.mybir as mybir
import concourse.tile as tile
from concourse.bass_utils import run_bass_kernel_spmd
from concourse.masks import make_identity

P = 128
S = 2048
D = 1024
GW = 512          # per-core head-group width (8 heads x 64)
DK = 64
HG = 8
NHP = HG // 2     # head pairs (even head rows 0-63, odd head rows 64-127)
NDC = D // P
NQT = S // 512
NSB = S // P
NCH = GW // P

F32 = mybir.dt.float32
BF = mybir.dt.bfloat16
SCALE = 0.125
N_CORES = 8


def vstart(kb, qt):
    # first causally-valid q in the 512-wide query tile for k-block kb
    return min(max(0, P * (kb - 4 * qt)), 384)


def build():
    nc = bacc.Bacc("TRN2", target_bir_lowering=False, debug=False)
    xb = nc.dram_tensor("xb", [S, D], BF, kind="ExternalInput")
    wq = nc.dram_tensor("wq", [D, GW], BF, kind="ExternalInput")
    wk = nc.dram_tensor("wk", [D, GW], BF, kind="ExternalInput")
    wv = nc.dram_tensor("wv", [D, GW], BF, kind="ExternalInput")
    wo = nc.dram_tensor("wo", [GW, D], BF, kind="ExternalInput")
    outp = nc.dram_tensor("outp", [S, D], BF, kind="ExternalOutput")

    with tile.TileContext(nc) as tc, \
         tc.tile_pool(name="const", bufs=1) as cpool, \
         tc.tile_pool(name="stores", bufs=1) as stores, \
         tc.tile_pool(name="wqkv", bufs=1) as wpool, \
         tc.tile_pool(name="xin", bufs=2) as xin, \
         tc.tile_pool(name="xt", bufs=2) as xtp, \
         tc.tile_pool(name="qtp", bufs=2) as qtp, \
         tc.tile_pool(name="e", bufs=24) as epool, \
         tc.tile_pool(name="lwork", bufs=1) as lwork, \
         tc.tile_pool(name="cstage", bufs=3) as cstage, \
         tc.tile_pool(name="ostage", bufs=3) as ostage, \
         tc.tile_pool(name="ps_sc", bufs=2, space="PSUM") as ps_sc, \
         tc.tile_pool(name="ps_cx", bufs=2, space="PSUM") as ps_cx, \
         tc.tile_pool(name="ps_pj", bufs=2, space="PSUM") as ps_pj:

        ident = cpool.tile([P, P], BF, tag="ident")
        make_identity(nc, ident[:])
        ones_bf = cpool.tile([P, DK], BF, tag="ones")
        nc.vector.tensor_copy(ones_bf[0:1, :],
                              nc.const_aps.tensor(1.0, (1, DK), F32))

        kT = stores.tile([P, NCH, S], BF, tag="kT")
        v_aug = stores.tile([P, NSB, HG, DK + 1], BF, tag="v")
        nc.vector.tensor_copy(
            v_aug[:, :, :, DK:DK + 1],
            nc.const_aps.tensor(1.0, (P, NSB, HG, 1), F32))
        ctx_l = stores.tile([P, NCH, S], BF, tag="ctxl")
        wo_t = stores.tile([P, NCH, D], BF, tag="wo")
        qT_tiles = {}
        xt_cur = {}

        # ---- projection emission units for one sequence-quarter ----------
        pstate = {}

        def load_wqkv():
            # on the ACT hwdge queue: parallel with x loads on sync
            wq_t = wpool.tile([P, NDC, GW], BF, tag="wq")
            wk_t = wpool.tile([P, NDC, GW], BF, tag="wk")
            wv_t = wpool.tile([P, NDC, GW], BF, tag="wv")
            # chunk 0 alone (unblocks the first K/Q projection), rest in one
            # DMA with 3x bigger descriptors
            for w_d, w_t in ((wk, wk_t), (wq, wq_t)):
                nc.scalar.dma_start(
                    w_t[:, :, 0:P],
                    w_d[:, 0:P].rearrange("(dc p) n -> p dc n", p=P))
                nc.scalar.dma_start(
                    w_t[:, :, P:GW],
                    w_d[:, P:GW].rearrange("(dc p) n -> p dc n", p=P))
            nc.scalar.dma_start(wv_t[:], wv.rearrange("(dc p) n -> p dc n", p=P))
            pstate["w"] = (wq_t, wk_t, wv_t)

        def load_wo():
            # deferred: not needed until the first out-projection (qt=1),
            # keeps the startup off the HBM critical path
            nc.scalar.dma_start(wo_t[:], wo.rearrange("(c p) n -> p c n", p=P))

        def p_start(q4):
            xt_cur[q4] = xtp.tile([P, NDC, 512], BF, tag="xt", name=f"xt{q4}")
            qT_tiles[q4] = qtp.tile([P, NCH, 512], BF, tag="qT", name=f"qT{q4}")

        def transpose_block(q4, sbl):
            xt_q = xt_cur[q4]
            sb = q4 * 4 + sbl
            x_blk = xin.tile([P, D], BF, tag="xin")
            nc.sync.dma_start(x_blk[:], xb[sb * P:(sb + 1) * P, :])
            tp_ps = ps_pj.tile([P, NDC, P], BF, tag="pj")
            for dc in range(NDC):
                nc.tensor.transpose(
                    tp_ps[:, dc, :], x_blk[:, dc * P:(dc + 1) * P], ident[:])
            nc.vector.tensor_copy(
                xt_q[:, :, sbl * P:(sbl + 1) * P], tp_ps[:])

        def qk_proj(q4, w_i, j, xt_q):
            w_t = pstate["w"][w_i]
            dst = qT_tiles[q4] if w_i == 0 else kT
            pj = ps_pj.tile([P, 512], F32, tag="pj")
            for dc in range(NDC):
                nc.tensor.matmul(pj[:], w_t[:, dc, j * P:(j + 1) * P],
                                 xt_q[:, dc, :],
                                 start=(dc == 0), stop=(dc == NDC - 1))
            if w_i == 0:
                nc.vector.tensor_copy(dst[:, j, :], pj[:])
            else:
                nc.vector.tensor_copy(
                    dst[:, j, q4 * 512:(q4 + 1) * 512], pj[:])

        def v_proj(q4, sbl, xt_q):
            sb = q4 * 4 + sbl
            pj = ps_pj.tile([P, 512], F32, tag="pj")
            for dc in range(NDC):
                nc.tensor.matmul(pj[:], xt_q[:, dc, sbl * P:(sbl + 1) * P],
                                 pstate["w"][2][:, dc, :],
                                 start=(dc == 0), stop=(dc == NDC - 1))
            nc.vector.tensor_copy(v_aug[:, sb, :, :DK], pj[:])

        def proj_units(q4):
            # interleaved K/Q/V per output chunk so woven consumers (scores
            # of head-pair j, ctx needing V) unblock as early as possible
            units = [lambda: p_start(q4)]
            units += [lambda sbl=sbl: transpose_block(q4, sbl) for sbl in range(4)]

            def mk(w_i, j):
                return lambda: qk_proj(q4, w_i, j, xt_cur[q4])

            def mkv(sbl):
                return lambda: v_proj(q4, sbl, xt_cur[q4])
            units += [mk(1, 0), mk(0, 0), mk(1, 1), mk(0, 1), mkv(0), mkv(1),
                      mk(1, 2), mk(0, 2), mk(1, 3), mk(0, 3), mkv(2), mkv(3)]
            return units

        # ---- attention group emitters (two heads 2hp, 2hp+1 at once) -----
        def emit_scores(hp, qt):
            e_blocks = []
            for kb in range(4 * (qt + 1)):
                vs = vstart(kb, qt)
                s_ps = ps_sc.tile([P, 2, 512], F32, tag="sc")
                for slot in range(2):
                    po = 64 * slot
                    nc.tensor.matmul(s_ps[:, slot, vs:],
                                     kT[po:po + 64, hp, kb * P:(kb + 1) * P],
                                     qT_tiles[qt][po:po + 64, hp, vs:],
                                     start=True, stop=True)
                e_sb = epool.tile([P, 2, 512], BF, tag="e")
                nc.scalar.activation(e_sb[:, :, vs:], s_ps[:, :, vs:],
                                     mybir.ActivationFunctionType.Exp,
                                     scale=SCALE)
                if kb >= 4 * qt:
                    # diagonal block: zero below-diagonal; in the 128-wide
                    # window starting at vs the mask is always keep j >= k
                    nc.gpsimd.affine_select(
                        out=e_sb[:, :, vs:vs + P], in_=e_sb[:, :, vs:vs + P],
                        compare_op=mybir.AluOpType.is_ge,
                        fill=0.0, base=0,
                        pattern=[[0, 2], [1, P]], channel_multiplier=-1)
                e_blocks.append((e_sb, vs))
            return e_blocks

        def emit_ctx(hp, qt, e_blocks, fill=()):
            nk = 4 * (qt + 1)
            final = fill != ()
            ctx_ps = [ps_cx.tile([P, 512], F32, tag="cx", name=f"cx{slot}")
                      for slot in range(2)]
            for kb in range(nk):
                e_sb, vs = e_blocks[kb]
                for slot in range(2):
                    nc.tensor.matmul(ctx_ps[slot][0:DK + 1, vs:],
                                     v_aug[:, kb, 2 * hp + slot, :],
                                     e_sb[:, slot, vs:],
                                     start=(kb == 0), stop=(kb == nk - 1),
                                     skip_group_check=True)
            for u in fill:
                u()   # PE filler that hides the final 1/l chain below
            if final:
                # fast tail chain: staging copies on DVE and ACT in
                # parallel, the two slots' l DMAs on separate queues,
                # 1/l broadcast on the (now idle) PE instead of gpsimd,
                # final muls on DVE and gpsimd in parallel
                dmae = (nc.sync, nc.scalar)
                mule = (nc.vector, nc.vector)   # gpsimd cannot read PSUM
                stgs, lrp = [], []
                for slot in range(2):
                    stg = cstage.tile([P, 512], BF, tag="stg")
                    if slot == 0:
                        nc.vector.tensor_copy(stg[0:DK + 1, :],
                                              ctx_ps[slot][0:DK + 1, :])
                    else:
                        nc.scalar.activation(stg[0:DK + 1, :],
                                             ctx_ps[slot][0:DK + 1, :],
                                             mybir.ActivationFunctionType.Copy)
                    stgs.append(stg)
                for slot in range(2):
                    lsp = lwork.tile([P, 512], BF, tag=f"fl{slot}")
                    dmae[slot].dma_start(lsp[0:8, 0:64],
                                         stgs[slot][DK:DK + 1, :])
                    lre = lwork.tile([P, 512], BF, tag=f"fr{slot}")
                    with nc.allow_low_precision(reason="1/l in bf16 is fine"):
                        nc.vector.reciprocal(lre[0:8, 0:64], lsp[0:8, 0:64])
                    linv = lwork.tile([P, 512], BF, tag=f"fi{slot}")
                    dmae[slot].dma_start(linv[0:1, :], lre[0:8, 0:64])
                    lrep_ps = ps_pj.tile([P, 512], F32, tag="pj")
                    nc.tensor.matmul(lrep_ps[0:DK, :], ones_bf[0:1, :],
                                     linv[0:1, :], start=True, stop=True)
                    lrp.append(lrep_ps)
                for slot in range(2):
                    mule[slot].tensor_mul(
                        out=ctx_l[64 * slot:64 * slot + DK, hp,
                                  qt * 512:(qt + 1) * 512],
                        in0=stgs[slot][0:DK, :], in1=lrp[slot][0:DK, :])
                return
            for slot in range(2):
                po = 64 * slot
                # stage ctx+l out of PSUM immediately to free the bank
                stg = cstage.tile([P, 512], BF, tag="stg")
                nc.vector.tensor_copy(stg[0:DK + 1, :], ctx_ps[slot][0:DK + 1, :])
                # 1/l: spread the 512 l values over 8 partitions so the DVE
                # reciprocal (serial within a partition) is cheap, gather
                # back, broadcast over the DK ctx partitions.
                lsp = lwork.tile([P, 512], BF, tag="lsp")
                nc.sync.dma_start(lsp[0:8, 0:64], stg[DK:DK + 1, :])
                lre = lwork.tile([P, 512], F32, tag="lre")
                nc.vector.reciprocal(lre[0:8, 0:64], lsp[0:8, 0:64])
                linv = lwork.tile([P, 512], F32, tag="linv")
                nc.sync.dma_start(linv[0:1, :], lre[0:8, 0:64])
                lrep = lwork.tile([P, 512], F32, tag="lrep")
                nc.gpsimd.partition_broadcast(lrep[0:DK, :], linv[0:1, :],
                                              channels=DK)
                nc.vector.tensor_mul(
                    out=ctx_l[po:po + DK, hp, qt * 512:(qt + 1) * 512],
                    in0=stg[0:DK, :], in1=lrep[0:DK, :])

        def out_group(qb, nh):
            po_ps = ps_pj.tile([P, 512], F32, tag="pj")
            for c in range(NCH):
                nc.tensor.matmul(
                    po_ps[:], ctx_l[:, c, qb * P:(qb + 1) * P],
                    wo_t[:, c, nh * 512:(nh + 1) * 512],
                    start=(c == 0), stop=(c == NCH - 1))
            ost = ostage.tile([P, 512], BF, tag="ost")
            nc.vector.tensor_copy(ost[:], po_ps[:])
            nc.sync.dma_start(
                outp[qb * P:(qb + 1) * P, nh * 512:(nh + 1) * 512],
                ost[:])

        # ---- interleaved emission ----------------------------------------
        # Startup prefix: only what the first attention group needs — x
        # quarter 0 transposed, W_k/W_q/W_v in flight, K/Q chunk 0
        # projected. The rest of quarter 0's projections weave into
        # attention quarter 0 alongside quarter 1's.
        p0 = proj_units(0)
        for u in p0[:5]:
            u()                      # p_start + 4 transpose blocks
        load_wqkv()
        p0[5]()                      # K proj chunk 0
        p0[6]()                      # Q proj chunk 0
        # emission order is dependency-binding order: all V units must be
        # emitted before the first emit_ctx (slot hp1), K/Q chunk j before
        # scores of head-pair j
        p0_rest = [p0[9], p0[10], p0[15], p0[16],          # V0..V3
                   p0[7], p0[8], p0[11], p0[12], p0[13], p0[14]]  # K/Q 1..3

        prev = None
        for qt in range(NQT):
            pu = list(p0_rest) if qt == 0 else []
            p0_rest = []
            if qt < NQT - 1:
                pu += proj_units(qt + 1)
            if qt == 0:
                pu.append(load_wo)
            held = []
            if qt >= 1:
                # output projection for quarter qt-1 (ctx complete once the
                # last head-pair of qt-1 is emitted, at hp=0 below); at
                # qt=3 hold half back as PE filler for the final 1/l chain
                oq = qt - 1
                og = [(lambda qb=qb, nh=nh: out_group(qb, nh))
                      for qb in range(4 * oq, 4 * oq + 4) for nh in range(2)]
                if qt == NQT - 1:
                    pu += og[:4]
                    held = og[4:]
                else:
                    pu += og
            pi = 0
            for hp in range(NHP):
                e_blocks = emit_scores(hp, qt)
                if prev is not None:
                    emit_ctx(*prev)
                prev = (hp, qt, e_blocks)
                take = ((hp + 1) * len(pu)) // NHP - (hp * len(pu)) // NHP
                for _ in range(take):
                    pu[pi]()
                    pi += 1
        emit_ctx(*prev, fill=tuple(held))
        for qb in range(S // P - 4, S // P):   # out-proj for the last quarter
            for nh in range(2):
                out_group(qb, nh)
    nc.compile()
    return nc


_NC_CACHE = None


def _get_nc():
    global _NC_CACHE
    if _NC_CACHE is None:
        _NC_CACHE = build()
    return _NC_CACHE


def _run(x, W_q, W_k, W_v, W_o, trace=False, tmpdir=None):
    import ml_dtypes
    bf16 = ml_dtypes.bfloat16
    x = np.asarray(x).astype(bf16)
    W_q = np.asarray(W_q).astype(bf16)
    W_k = np.asarray(W_k).astype(bf16)
    W_v = np.asarray(W_v).astype(bf16)
    W_o = np.asarray(W_o).astype(bf16)
    B = x.shape[0]
    in_maps = []
    for c in range(N_CORES):
        b, g = c // 2, c % 2
        in_maps.append({
            "xb": np.ascontiguousarray(x[b]),
            "wq": np.ascontiguousarray(W_q[:, g * GW:(g + 1) * GW]),
            "wk": np.ascontiguousarray(W_k[:, g * GW:(g + 1) * GW]),
            "wv": np.ascontiguousarray(W_v[:, g * GW:(g + 1) * GW]),
            "wo": np.ascontiguousarray(W_o[g * GW:(g + 1) * GW, :]),
        })
    nc = _get_nc()
    res = run_bass_kernel_spmd(nc, in_maps, core_ids=list(range(N_CORES)),
                               trace=trace, tmpdir=tmpdir)
    out = np.empty((B, S, D), np.float32)
    for b in range(B):
        out[b] = (res.results[2 * b]["outp"].astype(np.float32)
                  + res.results[2 * b + 1]["outp"].astype(np.float32))
    return out, res


def kernel(x, W_q, W_k, W_v, W_o):
    out, _ = _run(x, W_q, W_k, W_v, W_o)
    return out
